# revision 1
# baseline (speedup 1.0000x reference)
"""Windowed cross-attention with relative position encodings, data-parallel
over batch across 8 NeuronCores.

Sharding (per spec hint): B=32 is split 4-per-core across the 8 cores;
the small q/kv/proj weights and the 169x1152 RPE table are replicated.
Windows are independent so attention needs no cross-device communication.

The RPE gather (static 169 -> [49,49] index table) is folded on the host
into dense per-(i,j,head) tables so each core runs pure einsum/softmax work.
"""

import functools

import numpy as np

import jax
import jax.numpy as jnp

WS = 7
NH = 12
DIM = 384
HD = DIM // NH
L = WS * WS
SCALE = HD ** (-0.5)
N_CORES = 8


def _relative_position_index() -> np.ndarray:
    coords = np.stack(np.meshgrid(np.arange(WS), np.arange(WS), indexing="ij"))
    flat = coords.reshape(2, -1)
    rel = flat[:, :, None] - flat[:, None, :]
    rel = rel.transpose(1, 2, 0).copy()
    rel[:, :, 0] += WS - 1
    rel[:, :, 1] += WS - 1
    rel[:, :, 0] *= 2 * WS - 1
    return rel.sum(-1)  # [L, L] int


_RPI = _relative_position_index()


def _partition(t, b, h, w):
    nh, nw = h // WS, w // WS
    t = t.reshape(b, nh, WS, nw, WS, NH, HD)
    t = t.transpose(0, 1, 3, 5, 2, 4, 6)
    return t.reshape(b * nh * nw, NH, L, HD)


def _unpartition(t, b, h, w):
    nh, nw = h // WS, w // WS
    t = t.reshape(b, nh, nw, NH, WS, WS, HD)
    t = t.transpose(0, 1, 4, 2, 5, 3, 6)
    return t.reshape(b, h, w, DIM)


def _core_fn(x, context, q_w, q_b, kv_w, kv_b, proj_w, proj_b,
             q_rpe, k_rpe, v_rpe):
    b, h, w, _ = x.shape
    q = x @ q_w + q_b
    kv = context @ kv_w + kv_b
    k, v = jnp.split(kv, 2, axis=-1)

    q = _partition(q, b, h, w) * SCALE
    k = _partition(k, b, h, w)
    v = _partition(v, b, h, w)

    qk = jnp.einsum("bhic,bhjc->bhij", q, k)
    qr = jnp.einsum("bhic,ijhc->bhij", q, k_rpe)
    kr = jnp.einsum("bhjc,ijhc->bhij", k, q_rpe)
    attn = jax.nn.softmax(qk + qr + kr, axis=-1)

    out = jnp.einsum("bhij,bhjc->bhic", attn, v) + jnp.einsum(
        "bhij,ijhc->bhic", attn, v_rpe
    )
    out = _unpartition(out, b, h, w)
    return out @ proj_w + proj_b


_PMAP = None


def _get_pmap():
    global _PMAP
    if _PMAP is None:
        _PMAP = jax.pmap(
            _core_fn,
            in_axes=(0, 0, None, None, None, None, None, None, None, None, None),
            devices=jax.devices()[:N_CORES],
        )
    return _PMAP


def kernel(x, context, rpe_table, q_w, q_b, kv_w, kv_b, proj_w, proj_b):
    x = np.asarray(x)
    context = np.asarray(context)
    B, H, W, _ = x.shape
    per = B // N_CORES

    # host-side fold of the static gather: [169, 1152] -> three [L,L,NH,HD]
    rpe = np.asarray(rpe_table)[_RPI.reshape(-1)].reshape(L, L, NH, 3 * HD)
    q_rpe, k_rpe, v_rpe = np.split(rpe, 3, axis=-1)
    q_rpe = (q_rpe * SCALE).astype(np.float32)
    k_rpe = np.ascontiguousarray(k_rpe, dtype=np.float32)
    v_rpe = np.ascontiguousarray(v_rpe, dtype=np.float32)

    xs = x.reshape(N_CORES, per, H, W, DIM)
    cs = context.reshape(N_CORES, per, H, W, DIM)

    out = _get_pmap()(
        xs, cs,
        np.asarray(q_w), np.asarray(q_b),
        np.asarray(kv_w), np.asarray(kv_b),
        np.asarray(proj_w), np.asarray(proj_b),
        q_rpe, k_rpe, v_rpe,
    )
    out = np.asarray(out).reshape(B, H, W, DIM)
    return out.astype(np.float32)
